# revision 25
# baseline (speedup 1.0000x reference)
"""Trainium2 Bass kernel for a 12-layer dense MLP (dims
2-10-20-50-100-200-1000-200-100-50-20-10-2, ReLU after every layer,
softmax over the final 2 logits), data-parallel over 8 NeuronCores.

Layout: feature-major. Activations live in SBUF as [features(partitions),
batch(free)]; weights W[fan_in, fan_out] are the matmul stationary operand
(lhsT), so each layer is psum[M, F] = W.T @ h[K, F]. Matmuls run in bf16
with fp32 PSUM accumulation.

Two half-width supertile streams (even / odd supertiles) are emitted
interleaved at (layer, evac-group) unit granularity, phase-shifted by half
a supertile: one stream is always in its dense 200x1000/1000x200 phase
while the other runs its small serial layers, so the PE array never idles
(and never triggers the HAM activity throttle), and each stream's
ReLU-evacuation latencies are covered by the other stream's matmuls.

Small layers' activations are packed into shared 128-partition tiles at
32-aligned bases (tile_position routes matmuls to matching PE array
tiles). ReLU+bias evacuations are [mc, 1024] instructions balanced
between ScalarE and VectorE. softmax([a,b]) == [sigmoid(a-b),
sigmoid(b-a)] is a 2x2 difference matmul + Sigmoid.
"""

import ml_dtypes
import numpy as np

import concourse.bass as bass
import concourse.mybir as mybir
import concourse.tile as tile
from concourse import bacc
from concourse.bass_utils import run_bass_kernel_spmd

DIMS = [2, 10, 20, 50, 100, 200, 1000, 200, 100, 50, 20, 10, 2]
N_CORES = 8
N = 262144
B = N // N_CORES   # batch per core (32768)
F = 512            # columns per matmul (PSUM bank, fp32)
SW = 2048          # supertile width per stream
ST = B // SW       # supertiles per core (16)
EG = 1024          # evac group width
GPB = EG // F      # 512-col blocks per evac group (2)
G = SW // EG       # evac groups per supertile (2)

F32 = mybir.dt.float32
BF16 = mybir.dt.bfloat16

N_LAYERS = len(DIMS) - 1  # 12


def _chunks(n: int, maxc: int = 128) -> list[tuple[int, int]]:
    num = -(-n // maxc)
    size = -(-n // num)
    out = []
    s = 0
    while s < n:
        c = min(size, n - s)
        out.append((s, c))
        s += c
    return out


# h placement: (htensor tag, partition base) for each layer's output
# chunks. Lifetime-disjoint layers share a tensor tag. X1 packs h1/h2/h3
# (+h12), X2 packs h9/h10/h11. h4/h8 share "h100"; h5/h7 share "h200_*".
H_PLACE = {
    1: [("X1", 0)],
    2: [("X1", 32)],
    3: [("X1", 64)],
    4: [("h100", 0)],
    5: [("h200_0", 0), ("h200_1", 0)],
    6: [(f"h6_{m}", 0) for m in range(8)],
    7: [("h200_0", 0), ("h200_1", 0)],
    8: [("h100", 0)],
    9: [("X2", 0)],
    10: [("X2", 64)],
    11: [("X2", 96)],
    12: [("X1", 0)],
}
H_SIZE = {"X1": 128, "X2": 128, "h100": 100, "h200_0": 100, "h200_1": 100,
          **{f"h6_{m}": 125 for m in range(8)}}


def build_nc():
    nc = bacc.Bacc("TRN2", target_bir_lowering=False, debug=False,
                   num_devices=N_CORES)

    x_dram = nc.dram_tensor("xT", [DIMS[0], B], BF16,
                            kind="ExternalInput").ap()
    w_dram = [
        nc.dram_tensor(f"w{l}", [DIMS[l - 1], DIMS[l]], BF16,
                       kind="ExternalInput").ap()
        for l in range(1, N_LAYERS + 1)
    ]
    b_dram = [
        nc.dram_tensor(f"b{l}", [DIMS[l], 1], F32, kind="ExternalInput").ap()
        for l in range(1, N_LAYERS + 1)
    ]
    d_dram = nc.dram_tensor("D", [2, 2], BF16, kind="ExternalInput").ap()
    y_dram = nc.dram_tensor("y", [2, B], F32, kind="ExternalOutput").ap()

    eng_load = {"act": 0.0, "dve": 0.0}

    def _evac_act(out_ap, in_ap, bias_ap):
        nc.scalar.activation(out_ap, in_ap,
                             mybir.ActivationFunctionType.Relu, bias=bias_ap)

    def _evac_dve(out_ap, in_ap, bias_ap):
        nc.vector.tensor_scalar(out_ap, in_ap, bias_ap, 0.0,
                                mybir.AluOpType.add, mybir.AluOpType.max)

    def evac(out_ap, in_ap, bias_ap, pin_act=False, split=False):
        if split:
            # halve slot-return latency: both engines evacuate one half
            half = in_ap.shape[-1] // 2
            _evac_act(out_ap[:, :half], in_ap[:, :half], bias_ap)
            _evac_dve(out_ap[:, half:], in_ap[:, half:], bias_ap)
            eng_load["act"] += (half + 310) / 1.2
            eng_load["dve"] += (half + 205) / 0.96
            return
        act_cost = (EG + 310) / 1.2
        dve_cost = (EG + 205) / 0.96
        use_act = pin_act or (eng_load["act"] + act_cost
                              <= eng_load["dve"] + dve_cost)
        if use_act:
            eng_load["act"] += act_cost
            _evac_act(out_ap, in_ap, bias_ap)
        else:
            eng_load["dve"] += dve_cost
            _evac_dve(out_ap, in_ap, bias_ap)

    with tile.TileContext(nc) as tc:
        with (
            tc.tile_pool(name="wpool", bufs=1) as wpool,
            tc.tile_pool(name="hpool", bufs=1) as hpool,
            tc.tile_pool(name="iopool", bufs=2) as iopool,
            tc.tile_pool(name="psum", bufs=2, space="PSUM") as pspool,
        ):
            # ---- load weights/biases once, placed at their row bases ----
            wt = {}
            bt = {}
            rbase = {}
            for li in range(1, N_LAYERS + 1):
                rbase[li] = 0 if li == 1 else H_PLACE[li - 1][0][1]
            for li in range(1, N_LAYERS + 1):
                K, M = DIMS[li - 1], DIMS[li]
                rb = rbase[li]
                pad_fwl = li in (4, 5, 6, 7, 8)
                for ki, (ks, kc) in enumerate(_chunks(K)):
                    krb = rb if len(_chunks(K)) == 1 else 0
                    for mi, (ms, mc) in enumerate(_chunks(M)):
                        wcols = 128 if pad_fwl else mc
                        w = wpool.tile([krb + kc, wcols], BF16,
                                       name=f"wt{li}_{ki}_{mi}",
                                       tag=f"wt{li}_{ki}_{mi}", bufs=1)
                        if pad_fwl and mc < 128:
                            nc.vector.memset(w[krb:krb + kc, mc:], 0.0)
                        nc.sync.dma_start(
                            w[krb:krb + kc, :mc],
                            w_dram[li - 1][ks:ks + kc, ms:ms + mc])
                        wt[(li, ki, mi)] = (w[krb:krb + kc, :], mc,
                                            wcols)
                for mi, (ms, mc) in enumerate(_chunks(M)):
                    cb = H_PLACE[li][mi][1]
                    b = wpool.tile([cb + mc, 1], F32, name=f"bt{li}_{mi}",
                                   tag=f"bt{li}_{mi}", bufs=1)
                    nc.sync.dma_start(b[cb:cb + mc, :],
                                      b_dram[li - 1][ms:ms + mc, :])
                    bt[(li, mi)] = b[cb:cb + mc, 0:1]
            d_t = wpool.tile([2, 2], BF16, name="d_t", tag="d_t", bufs=1)
            nc.sync.dma_start(d_t[:], d_dram[:])

            def supertile_units(s, sfx):
                """Generator: emits one supertile's instructions, yielding
                at (layer, group) unit boundaries."""
                xt = iopool.tile([DIMS[0], SW], BF16, name=f"xt_{s}",
                                 tag=f"xt_{sfx}", bufs=2)
                nc.gpsimd.dma_start(xt[:], x_dram[:, s * SW:(s + 1) * SW])
                yield 0

                htiles = {}

                def htile(tag):
                    if tag not in htiles:
                        htiles[tag] = hpool.tile(
                            [H_SIZE[tag], SW], BF16, name=f"{tag}_{sfx}_{s}",
                            tag=f"{tag}_{sfx}", bufs=1)
                    return htiles[tag]

                hin = [(xt, 0)]
                for li in range(1, N_LAYERS + 1):
                    K, M = DIMS[li - 1], DIMS[li]
                    kch = _chunks(K)
                    mch = _chunks(M)
                    single_m = len(mch) == 1
                    for g in range(G):
                        for mi, (ms, mc) in enumerate(mch):
                            cb = H_PLACE[li][mi][1]
                            ps = pspool.tile([128, EG], F32,
                                             name=f"ps{li}_{mi}_{g}_{s}",
                                             tag=f"ps_{sfx}", bufs=2)
                            wap0, _, wcols = wt[(li, 0, mi)]
                            for ki, (ks, kc) in enumerate(kch):
                                rhs, rb = hin[ki]
                                wap = wt[(li, ki, mi)][0]
                                for f in range(GPB):
                                    c0 = g * EG + f * F
                                    nc.tensor.matmul(
                                        ps[cb:cb + wcols,
                                           f * F:(f + 1) * F],
                                        wap,
                                        rhs[rb:rb + kc, c0:c0 + F],
                                        start=(ki == 0),
                                        stop=(ki == len(kch) - 1),
                                        tile_position=(rb, cb),
                                    )
                            ht = htile(H_PLACE[li][mi][0])
                            evac(ht[cb:cb + mc, g * EG:(g + 1) * EG],
                                 ps[cb:cb + mc, :],
                                 bt[(li, mi)],
                                 pin_act=(single_m and (g == 0 or li == 12)),
                                 split=(li == 6))
                        yield len(mch) * len(kch) * GPB * 216
                    if single_m:
                        tag, cb = H_PLACE[li][0]
                        hin = [(htile(tag), cb)]
                    else:
                        hin = [(htile(H_PLACE[li][mi][0]), H_PLACE[li][mi][1])
                               for mi in range(len(mch))]

                # softmax: [sig(a-b), sig(b-a)] via 2x2 diff matmul + Sigmoid
                ot = iopool.tile([2, SW], F32, name=f"ot_{s}",
                                 tag=f"ot_{sfx}", bufs=2)
                h12, rb12 = hin[0]
                for g in range(G):
                    psd = pspool.tile([128, EG], F32, name=f"psd_{g}_{s}",
                                      tag=f"ps_{sfx}", bufs=2)
                    for f in range(GPB):
                        c0 = g * EG + f * F
                        nc.tensor.matmul(
                            psd[0:2, f * F:(f + 1) * F], d_t[:],
                            h12[rb12:rb12 + 2, c0:c0 + F],
                            start=True, stop=True, tile_position=(rb12, 0))
                    nc.scalar.activation(
                        ot[:, g * EG:(g + 1) * EG], psd[0:2, :],
                        mybir.ActivationFunctionType.Sigmoid)
                    yield GPB * 216
                nc.gpsimd.dma_start(y_dram[:, s * SW:(s + 1) * SW], ot[:])
                yield 0

            def stream(ss, sfx):
                for s in ss:
                    yield from supertile_units(s, sfx)

            # interleave two streams, time-balanced and antiphase: B's
            # emission position trails A's by half a supertile of PE time
            mm_per_st = sum(
                len(_chunks(DIMS[li - 1])) * len(_chunks(DIMS[li]))
                for li in range(1, N_LAYERS + 1)) * SW // F + SW // F
            half_st = mm_per_st * 216 // 4
            a = stream(range(0, ST, 2), "A")
            b = stream(range(1, ST, 2), "B")
            pos_a = 0
            pos_b = half_st
            a_done = b_done = False
            while not (a_done and b_done):
                if b_done or (not a_done and pos_a <= pos_b):
                    try:
                        pos_a += next(a)
                    except StopIteration:
                        a_done = True
                        pos_a = 10 ** 18
                else:
                    try:
                        pos_b += next(b)
                    except StopIteration:
                        b_done = True
                        pos_b = 10 ** 18

    nc.compile()
    return nc


_nc_cache = None


def _get_nc():
    global _nc_cache
    if _nc_cache is None:
        _nc_cache = build_nc()
    return _nc_cache


def _make_in_maps(x, Ws, bs):
    x = np.asarray(x, dtype=np.float32)
    Ws = [np.ascontiguousarray(
        np.asarray(w, dtype=np.float32).astype(ml_dtypes.bfloat16))
        for w in Ws]
    bs = [np.ascontiguousarray(np.asarray(b, dtype=np.float32).reshape(-1, 1))
          for b in bs]
    D = np.array([[1.0, -1.0], [-1.0, 1.0]], dtype=ml_dtypes.bfloat16)
    shared = {"D": D}
    for li in range(1, len(DIMS)):
        shared[f"w{li}"] = Ws[li - 1]
        shared[f"b{li}"] = bs[li - 1]
    in_maps = []
    for c in range(N_CORES):
        xT = np.ascontiguousarray(x[c * B:(c + 1) * B].T
                                  .astype(ml_dtypes.bfloat16))
        in_maps.append({"xT": xT, **shared})
    return in_maps


def run(x, Ws, bs, trace=False, **kw):
    nc = _get_nc()
    in_maps = _make_in_maps(x, Ws, bs)
    res = run_bass_kernel_spmd(nc, in_maps, core_ids=list(range(N_CORES)),
                               trace=trace, **kw)
    y = np.concatenate([r["y"].T for r in res.results], axis=0)
    return np.ascontiguousarray(y.astype(np.float32)), res


def kernel(x, Ws, bs):
    y, _ = run(x, Ws, bs, trace=False)
    return y


# revision 26
# speedup vs baseline: 1.1810x; 1.1810x over previous
"""Trainium2 Bass kernel for a 12-layer dense MLP (dims
2-10-20-50-100-200-1000-200-100-50-20-10-2, ReLU after every layer,
softmax over the final 2 logits), data-parallel over 8 NeuronCores.

Layout: feature-major. Activations live in SBUF as [features(partitions),
batch(free)]; weights W[fan_in, fan_out] are the matmul stationary operand
(lhsT), so each layer is psum[M, F] = W.T @ h[K, F]. Matmuls run in bf16
with fp32 PSUM accumulation.

Two half-width supertile streams (even / odd supertiles) are emitted
interleaved at (layer, evac-group) unit granularity, phase-shifted by half
a supertile: one stream is always in its dense 200x1000/1000x200 phase
while the other runs its small serial layers, so the PE array never idles
(and never triggers the HAM activity throttle), and each stream's
ReLU-evacuation latencies are covered by the other stream's matmuls.

Small layers' activations are packed into shared 128-partition tiles at
32-aligned bases (tile_position routes matmuls to matching PE array
tiles). ReLU+bias evacuations are [mc, 1024] instructions balanced
between ScalarE and VectorE. softmax([a,b]) == [sigmoid(a-b),
sigmoid(b-a)] is a 2x2 difference matmul + Sigmoid.
"""

import ml_dtypes
import numpy as np

import concourse.bass as bass
import concourse.mybir as mybir
import concourse.tile as tile
from concourse import bacc
from concourse.bass_utils import run_bass_kernel_spmd

DIMS = [2, 10, 20, 50, 100, 200, 1000, 200, 100, 50, 20, 10, 2]
N_CORES = 8
N = 262144
B = N // N_CORES   # batch per core (32768)
F = 512            # columns per matmul (PSUM bank, fp32)
SW = 2048          # supertile width per stream
ST = B // SW       # supertiles per core (16)
EG = 1024          # evac group width
GPB = EG // F      # 512-col blocks per evac group (2)
G = SW // EG       # evac groups per supertile (2)

F32 = mybir.dt.float32
BF16 = mybir.dt.bfloat16

N_LAYERS = len(DIMS) - 1  # 12


def _chunks(n: int, maxc: int = 128) -> list[tuple[int, int]]:
    num = -(-n // maxc)
    size = -(-n // num)
    out = []
    s = 0
    while s < n:
        c = min(size, n - s)
        out.append((s, c))
        s += c
    return out


# h placement: (htensor tag, partition base) for each layer's output
# chunks. Lifetime-disjoint layers share a tensor tag. X1 packs h1/h2/h3
# (+h12), X2 packs h9/h10/h11. h4/h8 share "h100"; h5/h7 share "h200_*".
H_PLACE = {
    1: [("X1", 0)],
    2: [("X1", 32)],
    3: [("X1", 64)],
    4: [("h100", 0)],
    5: [("h200_0", 0), ("h200_1", 0)],
    6: [(f"h6_{m}", 0) for m in range(8)],
    7: [("h200_0", 0), ("h200_1", 0)],
    8: [("h100", 0)],
    9: [("X2", 0)],
    10: [("X2", 64)],
    11: [("X2", 96)],
    12: [("X1", 0)],
}
H_SIZE = {"X1": 128, "X2": 128, "h100": 100, "h200_0": 100, "h200_1": 100,
          **{f"h6_{m}": 125 for m in range(8)}}


def build_nc():
    nc = bacc.Bacc("TRN2", target_bir_lowering=False, debug=False,
                   num_devices=N_CORES)

    x_dram = nc.dram_tensor("xT", [DIMS[0], B], BF16,
                            kind="ExternalInput").ap()
    w_dram = [
        nc.dram_tensor(f"w{l}", [DIMS[l - 1], DIMS[l]], BF16,
                       kind="ExternalInput").ap()
        for l in range(1, N_LAYERS + 1)
    ]
    b_dram = [
        nc.dram_tensor(f"b{l}", [DIMS[l], 1], F32, kind="ExternalInput").ap()
        for l in range(1, N_LAYERS + 1)
    ]
    d_dram = nc.dram_tensor("D", [2, 2], BF16, kind="ExternalInput").ap()
    y_dram = nc.dram_tensor("y", [2, B], F32, kind="ExternalOutput").ap()

    eng_load = {"act": 0.0, "dve": 0.0}

    def _evac_act(out_ap, in_ap, bias_ap):
        nc.scalar.activation(out_ap, in_ap,
                             mybir.ActivationFunctionType.Relu, bias=bias_ap)

    def _evac_dve(out_ap, in_ap, bias_ap):
        nc.vector.tensor_scalar(out_ap, in_ap, bias_ap, 0.0,
                                mybir.AluOpType.add, mybir.AluOpType.max)

    def evac(out_ap, in_ap, bias_ap, pin_act=False, split=False):
        if split:
            # halve slot-return latency: both engines evacuate one half
            half = in_ap.shape[-1] // 2
            _evac_act(out_ap[:, :half], in_ap[:, :half], bias_ap)
            _evac_dve(out_ap[:, half:], in_ap[:, half:], bias_ap)
            eng_load["act"] += (half + 310) / 1.2
            eng_load["dve"] += (half + 205) / 0.96
            return
        act_cost = (EG + 310) / 1.2
        dve_cost = (EG + 205) / 0.96
        use_act = pin_act or (eng_load["act"] + act_cost
                              <= eng_load["dve"] + dve_cost)
        if use_act:
            eng_load["act"] += act_cost
            _evac_act(out_ap, in_ap, bias_ap)
        else:
            eng_load["dve"] += dve_cost
            _evac_dve(out_ap, in_ap, bias_ap)

    with tile.TileContext(nc) as tc:
        with (
            tc.tile_pool(name="wpool", bufs=1) as wpool,
            tc.tile_pool(name="hpool", bufs=1) as hpool,
            tc.tile_pool(name="iopool", bufs=2) as iopool,
            tc.tile_pool(name="psum", bufs=2, space="PSUM") as pspool,
        ):
            # ---- load weights/biases once, placed at their row bases ----
            wt = {}
            bt = {}
            rbase = {}
            for li in range(1, N_LAYERS + 1):
                rbase[li] = 0 if li == 1 else H_PLACE[li - 1][0][1]
            for li in range(1, N_LAYERS + 1):
                K, M = DIMS[li - 1], DIMS[li]
                rb = rbase[li]
                pad_fwl = li in (4, 5, 6, 7, 8)
                for ki, (ks, kc) in enumerate(_chunks(K)):
                    krb = rb if len(_chunks(K)) == 1 else 0
                    for mi, (ms, mc) in enumerate(_chunks(M)):
                        wcols = 128 if pad_fwl else mc
                        w = wpool.tile([krb + kc, wcols], BF16,
                                       name=f"wt{li}_{ki}_{mi}",
                                       tag=f"wt{li}_{ki}_{mi}", bufs=1)
                        if pad_fwl and mc < 128:
                            nc.vector.memset(w[krb:krb + kc, mc:], 0.0)
                        nc.sync.dma_start(
                            w[krb:krb + kc, :mc],
                            w_dram[li - 1][ks:ks + kc, ms:ms + mc])
                        wt[(li, ki, mi)] = (w[krb:krb + kc, :], mc,
                                            wcols)
                for mi, (ms, mc) in enumerate(_chunks(M)):
                    cb = H_PLACE[li][mi][1]
                    b = wpool.tile([cb + mc, 1], F32, name=f"bt{li}_{mi}",
                                   tag=f"bt{li}_{mi}", bufs=1)
                    nc.sync.dma_start(b[cb:cb + mc, :],
                                      b_dram[li - 1][ms:ms + mc, :])
                    bt[(li, mi)] = b[cb:cb + mc, 0:1]
            d_t = wpool.tile([2, 2], BF16, name="d_t", tag="d_t", bufs=1)
            nc.sync.dma_start(d_t[:], d_dram[:])

            def supertile_units(s, sfx):
                """Generator: emits one supertile's instructions, yielding
                at (layer, group) unit boundaries."""
                xt = iopool.tile([DIMS[0], SW], BF16, name=f"xt_{s}",
                                 tag=f"xt_{sfx}", bufs=2)
                nc.gpsimd.dma_start(xt[:], x_dram[:, s * SW:(s + 1) * SW])
                yield 0

                htiles = {}

                def htile(tag):
                    if tag not in htiles:
                        htiles[tag] = hpool.tile(
                            [H_SIZE[tag], SW], BF16, name=f"{tag}_{sfx}_{s}",
                            tag=f"{tag}_{sfx}", bufs=1)
                    return htiles[tag]

                hin = [(xt, 0)]
                for li in range(1, N_LAYERS + 1):
                    K, M = DIMS[li - 1], DIMS[li]
                    kch = _chunks(K)
                    mch = _chunks(M)
                    single_m = len(mch) == 1
                    for g in range(G):
                        for mi, (ms, mc) in enumerate(mch):
                            cb = H_PLACE[li][mi][1]
                            ps = pspool.tile([128, EG], F32,
                                             name=f"ps{li}_{mi}_{g}_{s}",
                                             tag=f"ps_{sfx}", bufs=2)
                            wap0, _, wcols = wt[(li, 0, mi)]
                            for ki, (ks, kc) in enumerate(kch):
                                rhs, rb = hin[ki]
                                wap = wt[(li, ki, mi)][0]
                                for f in range(GPB):
                                    c0 = g * EG + f * F
                                    nc.tensor.matmul(
                                        ps[cb:cb + wcols,
                                           f * F:(f + 1) * F],
                                        wap,
                                        rhs[rb:rb + kc, c0:c0 + F],
                                        start=(ki == 0),
                                        stop=(ki == len(kch) - 1),
                                        tile_position=(rb, cb),
                                    )
                            ht = htile(H_PLACE[li][mi][0])
                            evac(ht[cb:cb + mc, g * EG:(g + 1) * EG],
                                 ps[cb:cb + mc, :],
                                 bt[(li, mi)],
                                 pin_act=(single_m and (g == 0 or li == 12)),
                                 split=(li == 6))
                        yield len(mch) * len(kch) * GPB * 216
                    if single_m:
                        tag, cb = H_PLACE[li][0]
                        hin = [(htile(tag), cb)]
                    else:
                        hin = [(htile(H_PLACE[li][mi][0]), H_PLACE[li][mi][1])
                               for mi in range(len(mch))]

                # softmax: [sig(a-b), sig(b-a)] via 2x2 diff matmul + Sigmoid
                ot = iopool.tile([2, SW], F32, name=f"ot_{s}",
                                 tag=f"ot_{sfx}", bufs=2)
                h12, rb12 = hin[0]
                for g in range(G):
                    psd = pspool.tile([128, EG], F32, name=f"psd_{g}_{s}",
                                      tag=f"ps_{sfx}", bufs=2)
                    for f in range(GPB):
                        c0 = g * EG + f * F
                        nc.tensor.matmul(
                            psd[0:2, f * F:(f + 1) * F], d_t[:],
                            h12[rb12:rb12 + 2, c0:c0 + F],
                            start=True, stop=True, tile_position=(rb12, 0))
                    nc.scalar.activation(
                        ot[:, g * EG:(g + 1) * EG], psd[0:2, :],
                        mybir.ActivationFunctionType.Sigmoid)
                    yield GPB * 216
                nc.gpsimd.dma_start(y_dram[:, s * SW:(s + 1) * SW], ot[:])
                yield 0

            def stream(ss, sfx):
                for s in ss:
                    yield from supertile_units(s, sfx)

            # interleave two streams, time-balanced and antiphase: B's
            # emission position trails A's by half a supertile of PE time
            mm_per_st = sum(
                len(_chunks(DIMS[li - 1])) * len(_chunks(DIMS[li]))
                for li in range(1, N_LAYERS + 1)) * SW // F + SW // F
            half_st = mm_per_st * 216 // 2
            a = stream(range(0, ST, 2), "A")
            b = stream(range(1, ST, 2), "B")
            pos_a = 0
            pos_b = half_st
            a_done = b_done = False
            while not (a_done and b_done):
                if b_done or (not a_done and pos_a <= pos_b):
                    try:
                        pos_a += next(a)
                    except StopIteration:
                        a_done = True
                        pos_a = 10 ** 18
                else:
                    try:
                        pos_b += next(b)
                    except StopIteration:
                        b_done = True
                        pos_b = 10 ** 18

    nc.compile()
    return nc


_nc_cache = None


def _get_nc():
    global _nc_cache
    if _nc_cache is None:
        _nc_cache = build_nc()
    return _nc_cache


def _make_in_maps(x, Ws, bs):
    x = np.asarray(x, dtype=np.float32)
    Ws = [np.ascontiguousarray(
        np.asarray(w, dtype=np.float32).astype(ml_dtypes.bfloat16))
        for w in Ws]
    bs = [np.ascontiguousarray(np.asarray(b, dtype=np.float32).reshape(-1, 1))
          for b in bs]
    D = np.array([[1.0, -1.0], [-1.0, 1.0]], dtype=ml_dtypes.bfloat16)
    shared = {"D": D}
    for li in range(1, len(DIMS)):
        shared[f"w{li}"] = Ws[li - 1]
        shared[f"b{li}"] = bs[li - 1]
    in_maps = []
    for c in range(N_CORES):
        xT = np.ascontiguousarray(x[c * B:(c + 1) * B].T
                                  .astype(ml_dtypes.bfloat16))
        in_maps.append({"xT": xT, **shared})
    return in_maps


def run(x, Ws, bs, trace=False, **kw):
    nc = _get_nc()
    in_maps = _make_in_maps(x, Ws, bs)
    res = run_bass_kernel_spmd(nc, in_maps, core_ids=list(range(N_CORES)),
                               trace=trace, **kw)
    y = np.concatenate([r["y"].T for r in res.results], axis=0)
    return np.ascontiguousarray(y.astype(np.float32)), res


def kernel(x, Ws, bs):
    y, _ = run(x, Ws, bs, trace=False)
    return y


# revision 27
# speedup vs baseline: 1.1993x; 1.0156x over previous
"""Trainium2 Bass kernel for a 12-layer dense MLP (dims
2-10-20-50-100-200-1000-200-100-50-20-10-2, ReLU after every layer,
softmax over the final 2 logits), data-parallel over 8 NeuronCores.

Layout: feature-major. Activations live in SBUF as [features(partitions),
batch(free)]; weights W[fan_in, fan_out] are the matmul stationary operand
(lhsT), so each layer is psum[M, F] = W.T @ h[K, F]. Matmuls run in bf16
with fp32 PSUM accumulation.

Two half-width supertile streams (even / odd supertiles) are emitted
interleaved at (layer, evac-group) unit granularity, phase-shifted by half
a supertile: one stream is always in its dense 200x1000/1000x200 phase
while the other runs its small serial layers, so the PE array never idles
(and never triggers the HAM activity throttle), and each stream's
ReLU-evacuation latencies are covered by the other stream's matmuls.

Small layers' activations are packed into shared 128-partition tiles at
32-aligned bases (tile_position routes matmuls to matching PE array
tiles). ReLU+bias evacuations are [mc, 1024] instructions balanced
between ScalarE and VectorE. softmax([a,b]) == [sigmoid(a-b),
sigmoid(b-a)] is a 2x2 difference matmul + Sigmoid.
"""

import ml_dtypes
import numpy as np

import concourse.bass as bass
import concourse.mybir as mybir
import concourse.tile as tile
from concourse import bacc
from concourse.bass_utils import run_bass_kernel_spmd

DIMS = [2, 10, 20, 50, 100, 200, 1000, 200, 100, 50, 20, 10, 2]
N_CORES = 8
N = 262144
B = N // N_CORES   # batch per core (32768)
F = 512            # columns per matmul (PSUM bank, fp32)
SW = 2048          # supertile width per stream
ST = B // SW       # supertiles per core (16)
EG = 1024          # evac group width
GPB = EG // F      # 512-col blocks per evac group (2)
G = SW // EG       # evac groups per supertile (2)

F32 = mybir.dt.float32
BF16 = mybir.dt.bfloat16

N_LAYERS = len(DIMS) - 1  # 12


def _chunks(n: int, maxc: int = 128) -> list[tuple[int, int]]:
    num = -(-n // maxc)
    size = -(-n // num)
    out = []
    s = 0
    while s < n:
        c = min(size, n - s)
        out.append((s, c))
        s += c
    return out


# h placement: (htensor tag, partition base) for each layer's output
# chunks. Lifetime-disjoint layers share a tensor tag. X1 packs h1/h2/h3
# (+h12), X2 packs h9/h10/h11. h4/h8 share "h100"; h5/h7 share "h200_*".
H_PLACE = {
    1: [("X1", 0)],
    2: [("X1", 32)],
    3: [("X1", 64)],
    4: [("h100", 0)],
    5: [("h200_0", 0), ("h200_1", 0)],
    6: [(f"h6_{m}", 0) for m in range(8)],
    7: [("h200_0", 0), ("h200_1", 0)],
    8: [("h100", 0)],
    9: [("X2", 0)],
    10: [("X2", 64)],
    11: [("X2", 96)],
    12: [("X1", 0)],
}
H_SIZE = {"X1": 128, "X2": 128, "h100": 100, "h200_0": 100, "h200_1": 100,
          **{f"h6_{m}": 125 for m in range(8)}}


def build_nc():
    nc = bacc.Bacc("TRN2", target_bir_lowering=False, debug=False,
                   num_devices=N_CORES)

    x_dram = nc.dram_tensor("xT", [DIMS[0], B], BF16,
                            kind="ExternalInput").ap()
    w_dram = [
        nc.dram_tensor(f"w{l}", [DIMS[l - 1], DIMS[l]], BF16,
                       kind="ExternalInput").ap()
        for l in range(1, N_LAYERS + 1)
    ]
    b_dram = [
        nc.dram_tensor(f"b{l}", [DIMS[l], 1], F32, kind="ExternalInput").ap()
        for l in range(1, N_LAYERS + 1)
    ]
    d_dram = nc.dram_tensor("D", [2, 2], BF16, kind="ExternalInput").ap()
    y_dram = nc.dram_tensor("y", [2, B], F32, kind="ExternalOutput").ap()

    eng_load = {"act": 0.0, "dve": 0.0}

    def _evac_act(out_ap, in_ap, bias_ap):
        nc.scalar.activation(out_ap, in_ap,
                             mybir.ActivationFunctionType.Relu, bias=bias_ap)

    def _evac_dve(out_ap, in_ap, bias_ap):
        nc.vector.tensor_scalar(out_ap, in_ap, bias_ap, 0.0,
                                mybir.AluOpType.add, mybir.AluOpType.max)

    def evac(out_ap, in_ap, bias_ap, pin_act=False, split=False):
        if split:
            # halve slot-return latency: both engines evacuate one half
            half = in_ap.shape[-1] // 2
            _evac_act(out_ap[:, :half], in_ap[:, :half], bias_ap)
            _evac_dve(out_ap[:, half:], in_ap[:, half:], bias_ap)
            eng_load["act"] += (half + 310) / 1.2
            eng_load["dve"] += (half + 205) / 0.96
            return
        act_cost = (EG + 310) / 1.2
        dve_cost = (EG + 205) / 0.96
        use_act = pin_act or (eng_load["act"] + act_cost
                              <= eng_load["dve"] + dve_cost)
        if use_act:
            eng_load["act"] += act_cost
            _evac_act(out_ap, in_ap, bias_ap)
        else:
            eng_load["dve"] += dve_cost
            _evac_dve(out_ap, in_ap, bias_ap)

    with tile.TileContext(nc) as tc:
        with (
            tc.tile_pool(name="wpool", bufs=1) as wpool,
            tc.tile_pool(name="hpool", bufs=1) as hpool,
            tc.tile_pool(name="iopool", bufs=2) as iopool,
            tc.tile_pool(name="psum", bufs=2, space="PSUM") as pspool,
        ):
            # ---- load weights/biases once, placed at their row bases ----
            wt = {}
            bt = {}
            rbase = {}
            for li in range(1, N_LAYERS + 1):
                rbase[li] = 0 if li == 1 else H_PLACE[li - 1][0][1]
            for li in range(1, N_LAYERS + 1):
                K, M = DIMS[li - 1], DIMS[li]
                rb = rbase[li]
                pad_fwl = li in (4, 5, 6, 7, 8)
                for ki, (ks, kc) in enumerate(_chunks(K)):
                    krb = rb if len(_chunks(K)) == 1 else 0
                    for mi, (ms, mc) in enumerate(_chunks(M)):
                        wcols = 128 if pad_fwl else mc
                        w = wpool.tile([krb + kc, wcols], BF16,
                                       name=f"wt{li}_{ki}_{mi}",
                                       tag=f"wt{li}_{ki}_{mi}", bufs=1)
                        if pad_fwl and mc < 128:
                            nc.vector.memset(w[krb:krb + kc, mc:], 0.0)
                        nc.sync.dma_start(
                            w[krb:krb + kc, :mc],
                            w_dram[li - 1][ks:ks + kc, ms:ms + mc])
                        wt[(li, ki, mi)] = (w[krb:krb + kc, :], mc,
                                            wcols)
                for mi, (ms, mc) in enumerate(_chunks(M)):
                    cb = H_PLACE[li][mi][1]
                    b = wpool.tile([cb + mc, 1], F32, name=f"bt{li}_{mi}",
                                   tag=f"bt{li}_{mi}", bufs=1)
                    nc.sync.dma_start(b[cb:cb + mc, :],
                                      b_dram[li - 1][ms:ms + mc, :])
                    bt[(li, mi)] = b[cb:cb + mc, 0:1]
            d_t = wpool.tile([2, 2], BF16, name="d_t", tag="d_t", bufs=1)
            nc.sync.dma_start(d_t[:], d_dram[:])

            def supertile_units(s, sfx):
                """Generator: emits one supertile's instructions, yielding
                at (layer, group) unit boundaries."""
                xt = iopool.tile([DIMS[0], SW], BF16, name=f"xt_{s}",
                                 tag=f"xt_{sfx}", bufs=2)
                nc.gpsimd.dma_start(xt[:], x_dram[:, s * SW:(s + 1) * SW])
                yield 0

                htiles = {}

                def htile(tag):
                    if tag not in htiles:
                        htiles[tag] = hpool.tile(
                            [H_SIZE[tag], SW], BF16, name=f"{tag}_{sfx}_{s}",
                            tag=f"{tag}_{sfx}", bufs=1)
                    return htiles[tag]

                hin = [(xt, 0)]
                for li in range(1, N_LAYERS + 1):
                    K, M = DIMS[li - 1], DIMS[li]
                    kch = _chunks(K)
                    mch = _chunks(M)
                    single_m = len(mch) == 1
                    for g in range(G):
                        for mi, (ms, mc) in enumerate(mch):
                            cb = H_PLACE[li][mi][1]
                            ps = pspool.tile([128, EG], F32,
                                             name=f"ps{li}_{mi}_{g}_{s}",
                                             tag=f"ps_{sfx}", bufs=2)
                            wap0, _, wcols = wt[(li, 0, mi)]
                            for ki, (ks, kc) in enumerate(kch):
                                rhs, rb = hin[ki]
                                wap = wt[(li, ki, mi)][0]
                                for f in range(GPB):
                                    c0 = g * EG + f * F
                                    nc.tensor.matmul(
                                        ps[cb:cb + wcols,
                                           f * F:(f + 1) * F],
                                        wap,
                                        rhs[rb:rb + kc, c0:c0 + F],
                                        start=(ki == 0),
                                        stop=(ki == len(kch) - 1),
                                        tile_position=(rb, cb),
                                    )
                            ht = htile(H_PLACE[li][mi][0])
                            evac(ht[cb:cb + mc, g * EG:(g + 1) * EG],
                                 ps[cb:cb + mc, :],
                                 bt[(li, mi)],
                                 pin_act=(single_m and g == 0),
                                 split=(li == 6))
                        yield len(mch) * len(kch) * GPB * 216
                    if single_m:
                        tag, cb = H_PLACE[li][0]
                        hin = [(htile(tag), cb)]
                    else:
                        hin = [(htile(H_PLACE[li][mi][0]), H_PLACE[li][mi][1])
                               for mi in range(len(mch))]

                # softmax: [sig(a-b), sig(b-a)] via 2x2 diff matmul + Sigmoid
                ot = iopool.tile([2, SW], F32, name=f"ot_{s}",
                                 tag=f"ot_{sfx}", bufs=2)
                h12, rb12 = hin[0]
                for g in range(G):
                    psd = pspool.tile([128, EG], F32, name=f"psd_{g}_{s}",
                                      tag=f"ps_{sfx}", bufs=2)
                    for f in range(GPB):
                        c0 = g * EG + f * F
                        nc.tensor.matmul(
                            psd[0:2, f * F:(f + 1) * F], d_t[:],
                            h12[rb12:rb12 + 2, c0:c0 + F],
                            start=True, stop=True, tile_position=(rb12, 0))
                    nc.scalar.activation(
                        ot[:, g * EG:(g + 1) * EG], psd[0:2, :],
                        mybir.ActivationFunctionType.Sigmoid)
                    yield GPB * 216
                nc.gpsimd.dma_start(y_dram[:, s * SW:(s + 1) * SW], ot[:])
                yield 0

            def stream(ss, sfx):
                for s in ss:
                    yield from supertile_units(s, sfx)

            # interleave two streams, time-balanced and antiphase: B's
            # emission position trails A's by half a supertile of PE time
            mm_per_st = sum(
                len(_chunks(DIMS[li - 1])) * len(_chunks(DIMS[li]))
                for li in range(1, N_LAYERS + 1)) * SW // F + SW // F
            half_st = mm_per_st * 216 // 2
            a = stream(range(0, ST, 2), "A")
            b = stream(range(1, ST, 2), "B")
            pos_a = 0
            pos_b = half_st
            a_done = b_done = False
            while not (a_done and b_done):
                if b_done or (not a_done and pos_a <= pos_b):
                    try:
                        pos_a += next(a)
                    except StopIteration:
                        a_done = True
                        pos_a = 10 ** 18
                else:
                    try:
                        pos_b += next(b)
                    except StopIteration:
                        b_done = True
                        pos_b = 10 ** 18

    nc.compile()
    return nc


_nc_cache = None


def _get_nc():
    global _nc_cache
    if _nc_cache is None:
        _nc_cache = build_nc()
    return _nc_cache


def _make_in_maps(x, Ws, bs):
    x = np.asarray(x, dtype=np.float32)
    Ws = [np.ascontiguousarray(
        np.asarray(w, dtype=np.float32).astype(ml_dtypes.bfloat16))
        for w in Ws]
    bs = [np.ascontiguousarray(np.asarray(b, dtype=np.float32).reshape(-1, 1))
          for b in bs]
    D = np.array([[1.0, -1.0], [-1.0, 1.0]], dtype=ml_dtypes.bfloat16)
    shared = {"D": D}
    for li in range(1, len(DIMS)):
        shared[f"w{li}"] = Ws[li - 1]
        shared[f"b{li}"] = bs[li - 1]
    in_maps = []
    for c in range(N_CORES):
        xT = np.ascontiguousarray(x[c * B:(c + 1) * B].T
                                  .astype(ml_dtypes.bfloat16))
        in_maps.append({"xT": xT, **shared})
    return in_maps


def run(x, Ws, bs, trace=False, **kw):
    nc = _get_nc()
    in_maps = _make_in_maps(x, Ws, bs)
    res = run_bass_kernel_spmd(nc, in_maps, core_ids=list(range(N_CORES)),
                               trace=trace, **kw)
    y = np.concatenate([r["y"].T for r in res.results], axis=0)
    return np.ascontiguousarray(y.astype(np.float32)), res


def kernel(x, Ws, bs):
    y, _ = run(x, Ws, bs, trace=False)
    return y
